# revision 3
# baseline (speedup 1.0000x reference)
"""Trainium2 Bass kernel for nn_LDRModel (scattering -> BN -> ReLU -> Toeplitz).

Approach: every FFT/conv step of the Kymatio scattering transform is a fixed
linear map, so we precompute dense matrices on the host and run the whole
network as a chain of matmuls + |z| nonlinearities on device. Since only
y[..., :10] of the Toeplitz product is kept, the final stage collapses to a
(feat x 40) matmul plus a tiny host-side (40 -> 10) combine.

Sharding: the 8 first-order wavelets (j1 axis) go one-per-core; every core
processes the full batch (384 = 3 channels x 128 images as matmul columns).
BatchNorm features land whole on one core, so stats are local (no
collectives). Per-core partial Toeplitz outputs (40 x 128) are summed on the
host.
"""

import os
import sys
import numpy as np

sys.path.insert(0, "/opt/trn_rl_repo")

import ml_dtypes

import concourse.bass as bass  # noqa: F401
import concourse.bacc as bacc
import concourse.mybir as mybir
import concourse.tile as tile
from concourse.bass_utils import run_bass_kernel_spmd
from concourse.kernels.tile_matmul import matmul_tile_kernel

BF16 = ml_dtypes.bfloat16
P = 128
NB = 384  # matmul free dim: 3 channels * 128 batch
NCORES = 8

# stage-1 output column blocks (padded to 128-row tiles)
#   [S0 64+64z | u1aRe 1600+64z | u1aIm 1600+64z | u1bRe 400+112z | u1bIm 400+112z]
S1_TILES = 35  # 1 + 13 + 13 + 4 + 4
# ucat rows: [u1aAbs 1664 | u1bAbs 512 | u2Abs 3200] -> 42 tiles
UCAT_TILES = 42
S2_TILES = 50  # stage2 out: Re 3200 | Im 3200
LOW_TILES = 5  # lowpass out: 640 = [S1a 64 | S1b 64 | S2 512]
F_TILES = 6  # f: [S0 tile | 5 lowpass tiles] = 768 rows/core (64 pad)

LAST = {}  # test.py introspection: BassKernelResults of last run


# ---------------------------------------------------------------------------
# host-side filter bank (numpy port of the reference, float64)
# ---------------------------------------------------------------------------

def _gabor(M, N, sigma, theta, xi, slant):
    R = np.array([[np.cos(theta), -np.sin(theta)], [np.sin(theta), np.cos(theta)]])
    D = np.array([[1.0, 0.0], [0.0, slant ** 2]])
    curv = R @ D @ R.T / (2.0 * sigma ** 2)
    xx, yy = np.mgrid[0:M, 0:N]
    gab = np.zeros((M, N), np.complex128)
    for ex in range(-2, 3):
        for ey in range(-2, 3):
            xg = xx + ex * M
            yg = yy + ey * N
            arg = -(curv[0, 0] * xg * xg + (curv[0, 1] + curv[1, 0]) * xg * yg
                    + curv[1, 1] * yg * yg) \
                + 1j * (xg * xi * np.cos(theta) + yg * xi * np.sin(theta))
            gab += np.exp(arg)
    return gab / (2.0 * np.pi * sigma ** 2 / slant)


def _morlet(M, N, sigma, theta, xi, slant):
    wv = _gabor(M, N, sigma, theta, xi, slant)
    env = _gabor(M, N, sigma, theta, 0.0, slant)
    return wv - (wv.sum() / env.sum()) * env


def _periodize(v, res):
    k = 2 ** res
    Mv, Nv = v.shape
    return v.reshape(k, Mv // k, k, Nv // k).mean(axis=(0, 2))


def _filters():
    Mp = 40
    phi = np.real(np.fft.fft2(_gabor(Mp, Mp, 0.8 * 2 ** 2, 0.0, 0.0, 1.0)))
    psis = {}
    for j in range(2):
        bank = []
        for t in range(8):
            theta = (3 - t) * np.pi / 8.0
            bank.append(np.real(np.fft.fft2(
                _morlet(Mp, Mp, 0.8 * 2 ** j, theta, 3.0 * np.pi / 4.0 / 2 ** j, 0.5))))
        psis[j] = np.stack(bank)
    return phi, _periodize(phi, 1), psis[0], psis[1]


def _unpad_sub(y, k):
    return y[..., ::k, ::k][..., 1:-1, 1:-1]


def _t3(a):
    """logical (R, N) -> stored (128, R//128, N) tile layout."""
    R, N = a.shape
    assert R % P == 0
    return np.ascontiguousarray(a.reshape(R // P, P, N).swapaxes(0, 1))


def _build_matrices():
    """All input-independent dense linear maps, as fp32/bf16 stored arrays."""
    phi0, phi1, psi0, psi1 = _filters()
    idx = np.pad(np.arange(32), 4, mode="reflect")

    # basis of padded input images
    E = np.zeros((1024, 32, 32))
    E[np.arange(1024), np.arange(1024) // 32, np.arange(1024) % 32] = 1.0
    Ep = E[:, idx][:, :, idx]                       # (1024,40,40)
    Xf = np.fft.fft2(Ep)

    s0map = _unpad_sub(np.fft.ifft2(Xf * phi0).real, 4).reshape(1024, 64)
    u1a = [np.fft.ifft2(Xf * psi0[k]).reshape(1024, 1600) for k in range(8)]
    u1b = [np.fft.ifft2(Xf * psi1[k])[:, ::2, ::2].reshape(1024, 400)
           for k in range(8)]

    E2 = np.eye(1600).reshape(1600, 40, 40)
    Xf2 = np.fft.fft2(E2)
    mphi0 = _unpad_sub(np.fft.ifft2(Xf2 * phi0).real, 4).reshape(1600, 64)
    m3 = [np.fft.ifft2(Xf2 * psi1[j])[:, ::2, ::2].reshape(1600, 400)
          for j in range(8)]

    E3 = np.eye(400).reshape(400, 20, 20)
    Xf3 = np.fft.fft2(E3)
    mphi1 = _unpad_sub(np.fft.ifft2(Xf3 * phi1).real, 2).reshape(400, 64)

    # per-core stage-1 weights (1024, 4480)
    w1_cores = []
    for k in range(8):
        w1 = np.zeros((1024, 4480), np.float32)
        w1[:, 0:64] = s0map
        w1[:, 128:1728] = u1a[k].real
        w1[:, 1792:3392] = u1a[k].imag
        w1[:, 3456:3856] = u1b[k].real
        w1[:, 3968:4368] = u1b[k].imag
        w1_cores.append(_t3(w1.astype(BF16)))

    # stage-2 weights (1664, 6400): K rows = u1aAbs (incl 64 pad), cols Re|Im
    w3 = np.zeros((1664, 6400), np.float32)
    for j in range(8):
        w3[:1600, 400 * j:400 * (j + 1)] = m3[j].real
        w3[:1600, 3200 + 400 * j:3200 + 400 * (j + 1)] = m3[j].imag
    w3 = _t3(w3.astype(BF16))

    # lowpass block-diagonal (5376, 640)
    wlow = np.zeros((5376, 640), np.float32)
    wlow[0:1600, 0:64] = mphi0
    wlow[1664:2064, 64:128] = mphi1
    for j in range(8):
        wlow[2176 + 400 * j:2176 + 400 * (j + 1), 128 + 64 * j:192 + 64 * j] = mphi1
    wlow = _t3(wlow.astype(BF16))

    # global feature index per core-row (768 rows/core, -1 = pad), per channel
    # row layout: m=0: [S0 64 | pad 64]; m=1..5: lowpass rows
    #   lowpass row: [S1a(q=1+k) 64 | S1b(q=9+k) 64 | S2(q=17+8k+j2) 64*8]
    gidx_cores = []
    for k in range(8):
        q_of_row = np.full(768, -1, np.int64)
        q_of_row[0:64] = 0
        q_of_row[128:192] = 1 + k
        q_of_row[192:256] = 9 + k
        for j in range(8):
            q_of_row[256 + 64 * j:320 + 64 * j] = 17 + 8 * k + j
        s_of_row = np.arange(768) % 64
        g = np.where(q_of_row >= 0, q_of_row * 64 + s_of_row, -1)  # (768,)
        # global feature = c*81*64 + g
        gi = np.where(g[None, :] >= 0, np.arange(3)[:, None] * 81 * 64 + g[None, :], -1)
        gidx_cores.append(gi)  # (3, 768)

    return dict(w1=w1_cores, w3=w3, wlow=wlow, gidx=gidx_cores)


# ---------------------------------------------------------------------------
# device program
# ---------------------------------------------------------------------------

def _build_program():
    f32, bf16 = mybir.dt.float32, mybir.dt.bfloat16
    nc = bacc.Bacc("TRN2", target_bir_lowering=False, debug=False,
                   enable_asserts=False, num_devices=NCORES)

    x_d = nc.dram_tensor("x", [P, 8, NB], bf16, kind="ExternalInput")
    w1_d = nc.dram_tensor("w1", [P, 8, 4480], bf16, kind="ExternalInput")
    w3_d = nc.dram_tensor("w3", [P, 13, 6400], bf16, kind="ExternalInput")
    wlow_d = nc.dram_tensor("wlow", [P, UCAT_TILES, 640], bf16, kind="ExternalInput")
    wt_d = nc.dram_tensor("wt", [P, F_TILES * 3, 40], bf16, kind="ExternalInput")
    ga_d = nc.dram_tensor("ga", [P, F_TILES * 3], mybir.dt.float32, kind="ExternalInput")
    be_d = nc.dram_tensor("be", [P, F_TILES * 3], mybir.dt.float32, kind="ExternalInput")
    t_out = nc.dram_tensor("t_out", [40, P], mybir.dt.float32, kind="ExternalOutput")

    s1out = nc.dram_tensor("s1out", [P, S1_TILES, NB], bf16)
    ucat = nc.dram_tensor("ucat", [P, UCAT_TILES, NB], bf16)
    s2out = nc.dram_tensor("s2out", [P, S2_TILES, NB], bf16)
    lowout = nc.dram_tensor("lowout", [P, LOW_TILES, NB], mybir.dt.float32)

    RELU = mybir.ActivationFunctionType.Relu
    SQRT = mybir.ActivationFunctionType.Sqrt
    AX = mybir.AxisListType.X

    with tile.TileContext(nc) as tc:
        # ---- stage 1: (1024 x 4480)^T @ (1024 x 384) -> s1out
        matmul_tile_kernel(tc, w1_d.ap(), x_d.ap(), s1out.ap())

        # ---- |z| for u1a (13 tiles) and u1b (4 tiles) -> ucat tiles 0..16
        def abs_stage(pool, src, pairs):
            for (mre, mim, mdst) in pairs:
                re = pool.tile([P, NB], bf16, tag="re")
                nc.sync.dma_start(re[:], src.ap()[:, mre, :])
                im = pool.tile([P, NB], bf16, tag="im")
                nc.sync.dma_start(im[:], src.ap()[:, mim, :])
                sq = pool.tile([P, NB], mybir.dt.float32, tag="sq")
                sq2 = pool.tile([P, NB], mybir.dt.float32, tag="sq2")
                nc.vector.tensor_mul(sq[:], re[:], re[:])
                nc.vector.tensor_mul(sq2[:], im[:], im[:])
                nc.vector.tensor_add(sq[:], sq[:], sq2[:])
                ab = pool.tile([P, NB], bf16, tag="ab")
                nc.scalar.activation(ab[:], sq[:], SQRT)
                nc.sync.dma_start(ucat.ap()[:, mdst, :], ab[:])

        with tc.tile_pool(name="abs1", bufs=3) as pool:
            abs_stage(pool, s1out,
                      [(1 + i, 14 + i, i) for i in range(13)]
                      + [(27 + i, 31 + i, 13 + i) for i in range(4)])

        # ---- stage 2: (1664 x 6400)^T @ u1aAbs (1664 x 384) -> s2out
        matmul_tile_kernel(tc, w3_d.ap(), ucat.ap()[:, 0:13, :], s2out.ap())

        # ---- |z| for u2 (25 tiles) -> ucat tiles 17..41
        with tc.tile_pool(name="abs2", bufs=3) as pool:
            abs_stage(pool, s2out, [(i, 25 + i, 17 + i) for i in range(25)])

        # ---- lowpass: (5376 x 640)^T @ ucat -> lowout (fp32)
        matmul_tile_kernel(tc, wlow_d.ap(), ucat.ap(), lowout.ap())

        # ---- BN + ReLU + Toeplitz partials
        with tc.tile_pool(name="fin", bufs=1) as fin, \
             tc.tile_pool(name="fwork", bufs=4) as fw, \
             tc.tile_pool(name="fpsum", bufs=1, space="PSUM") as pp:
            wt_sb = fin.tile([P, F_TILES * 3, 40], bf16)
            nc.sync.dma_start(wt_sb[:], wt_d.ap())
            ga_sb = fin.tile([P, F_TILES * 3], mybir.dt.float32)
            nc.sync.dma_start(ga_sb[:], ga_d.ap())
            be_sb = fin.tile([P, F_TILES * 3], mybir.dt.float32)
            nc.sync.dma_start(be_sb[:], be_d.ap())
            psum_t = pp.tile([40, P], mybir.dt.float32)

            nmm = 0
            for m in range(F_TILES):
                fsrc = fw.tile([P, NB], mybir.dt.float32, tag="fsrc")
                if m == 0:
                    fb = fw.tile([P, NB], bf16, tag="fb")
                    nc.sync.dma_start(fb[:], s1out.ap()[:, 0, :])
                    nc.vector.tensor_copy(fsrc[:], fb[:])
                else:
                    nc.sync.dma_start(fsrc[:], lowout.ap()[:, m - 1, :])
                sq = fw.tile([P, NB], mybir.dt.float32, tag="sq")
                nc.vector.tensor_mul(sq[:], fsrc[:], fsrc[:])
                for c in range(3):
                    cs = slice(c * P, (c + 1) * P)
                    col = m * 3 + c
                    st = fw.tile([P, 8], mybir.dt.float32, tag="st")
                    nc.vector.reduce_sum(st[:, 0:1], fsrc[:, cs], axis=AX)
                    nc.vector.reduce_sum(st[:, 1:2], sq[:, cs], axis=AX)
                    # mu = sum/128 ; msq = sumsq/128
                    nc.vector.tensor_scalar_mul(st[:, 2:3], st[:, 0:1], 1.0 / P)
                    nc.vector.tensor_scalar_mul(st[:, 3:4], st[:, 1:2], 1.0 / P)
                    # var = msq - mu^2
                    nc.vector.tensor_mul(st[:, 4:5], st[:, 2:3], st[:, 2:3])
                    nc.vector.tensor_sub(st[:, 5:6], st[:, 3:4], st[:, 4:5])
                    nc.vector.tensor_scalar_add(st[:, 5:6], st[:, 5:6], 1e-5)
                    # rstd = 1/sqrt(var + 1e-5)
                    nc.scalar.activation(st[:, 6:7], st[:, 5:6], SQRT)
                    nc.vector.reciprocal(st[:, 7:8], st[:, 6:7])
                    # scale = gamma * rstd ; bias = beta - mu * scale
                    sc = fw.tile([P, 2], mybir.dt.float32, tag="sc")
                    nc.vector.tensor_mul(sc[:, 0:1], ga_sb[:, col:col + 1], st[:, 7:8])
                    nc.vector.tensor_mul(st[:, 4:5], st[:, 2:3], sc[:, 0:1])
                    nc.vector.tensor_sub(sc[:, 1:2], be_sb[:, col:col + 1], st[:, 4:5])
                    fbn = fw.tile([P, P], bf16, tag="fbn")
                    nc.scalar.activation(fbn[:], fsrc[:, cs], RELU,
                                         bias=sc[:, 1:2], scale=sc[:, 0:1])
                    nmm += 1
                    nc.tensor.matmul(psum_t[:], wt_sb[:, col, :], fbn[:],
                                     start=(nmm == 1), stop=(nmm == F_TILES * 3))
            res_sb = fin.tile([40, P], mybir.dt.float32)
            nc.vector.tensor_copy(res_sb[:], psum_t[:])
            nc.sync.dma_start(t_out.ap()[:, :], res_sb[:])

    nc.compile()
    return nc


_STATE = {}


def _get_state():
    if "nc" not in _STATE:
        _STATE.update(_build_matrices())
        _STATE["nc"] = _build_program()
    return _STATE


def kernel(x, gamma, beta, G, H):
    st = _get_state()
    x = np.asarray(x, np.float32)
    gamma = np.asarray(gamma, np.float32)
    beta = np.asarray(beta, np.float32)
    G = np.asarray(G, np.float32)
    H = np.asarray(H, np.float32)

    # x (128,3,32,32) -> (1024 pix, 3*128) -> tiles
    xt = np.ascontiguousarray(x.transpose(2, 3, 1, 0).reshape(1024, NB))
    x_dev = _t3(xt.astype(BF16))

    # Toeplitz columns: Wt[feat, i*10+j] = H[i, feat-j]
    n = 15552
    wt_full = np.zeros((n, 40), np.float32)
    for i in range(4):
        for j in range(10):
            wt_full[j:, i * 10 + j] = H[i, :n - j]

    in_maps = []
    for k in range(NCORES):
        gi = st["gidx"][k]  # (3, 768)
        valid = gi >= 0
        gis = np.where(valid, gi, 0)
        ga = np.where(valid, gamma[gis], 0.0).astype(np.float32)   # (3,768)
        be = np.where(valid, beta[gis], 0.0).astype(np.float32)
        wt = np.where(valid[:, :, None], wt_full[gis], 0.0)        # (3,768,40)
        wt[:, 0:64, :] /= NCORES  # S0 rows replicated on every core
        # device layouts: ga/be (128, 18) with col = m*3+c ; wt (128, 18, 40)
        ga_dev = np.ascontiguousarray(
            ga.reshape(3, 6, P).transpose(2, 1, 0).reshape(P, 18))
        be_dev = np.ascontiguousarray(
            be.reshape(3, 6, P).transpose(2, 1, 0).reshape(P, 18))
        wt_dev = np.ascontiguousarray(
            wt.reshape(3, 6, P, 40).transpose(2, 1, 0, 3).reshape(P, 18, 40))
        in_maps.append({
            "x": x_dev,
            "w1": st["w1"][k],
            "w3": st["w3"],
            "wlow": st["wlow"],
            "wt": wt_dev.astype(BF16),
            "ga": ga_dev,
            "be": be_dev,
        })

    res = run_bass_kernel_spmd(st["nc"], in_maps, list(range(NCORES)), trace=False)
    LAST["res"] = res

    t = np.zeros((40, P), np.float64)
    for k in range(NCORES):
        t += res.results[k]["t_out"].astype(np.float64)

    # y[b, kk] = sum_{i, j<=kk} G[i, kk-j] * t[i*10+j, b]
    C = np.zeros((40, 10), np.float64)
    for i in range(4):
        for j in range(10):
            C[i * 10 + j, j:] = G[i, :10 - j]
    y = (t.T @ C).astype(np.float32)
    return y
